# revision 4
# baseline (speedup 1.0000x reference)
"""Trainium2 Bass kernel for nn_DecoderFactoredLSTM.

Model (reference.py): captions -> embedding -> per-gate affine chain
(V -> Sf -> U, no recurrence) -> 40-step LSTM over T with h @ W_g^T ->
concat hiddens -> C projection to vocab.

Strategy:
  * Host folds the input-dependent affine chain: u_g = x @ (U_g S_g V_g)^T + b,
    evaluated per unique caption token, expanded to u_all [T*B, 4H] (bf16).
  * Device (8 cores, SPMD, no collectives):
      - replicated LSTM recurrence (PE-streams Wcat.T each step),
      - h transposed on PE into a resident hiddens^T slab,
      - C projection sharded over vocab (4000 rows/core), interleaved with
        the recurrence per 8-step chunk.
  * Host reassembles [T*B, V] from the per-core [4000, T*B] outputs.
"""

import os
import time

import numpy as np
import ml_dtypes

import jax

import concourse.bacc as bacc
import concourse.mybir as mybir
import concourse.tile as tile
from concourse import bass, bass2jax
from concourse.masks import make_identity

BF16 = ml_dtypes.bfloat16

# problem constants (hardcoded per contract)
V, E, H, F, B, T = 32000, 512, 1024, 512, 64, 40
NCORES = 8
VS = V // NCORES          # vocab shard per core = 4000
MT = 125                  # vocab M-tile size
NM = VS // MT             # 32 m-tiles per core
G4 = 4 * H                # 4096 fused gate width
KT = H // 128             # 8 contraction tiles
TB = T * B                # 2560 rows
TCH = 8                   # recurrence steps per projection chunk
NTAU = T // TCH           # 5 projection chunks

f32 = mybir.dt.float32
bf16 = mybir.dt.bfloat16

Sig = mybir.ActivationFunctionType.Sigmoid
Tanh = mybir.ActivationFunctionType.Tanh
Ident = mybir.ActivationFunctionType.Identity


# ---------------------------------------------------------------- device code
def build_nc(reps: int = 1):
    nc = bacc.Bacc(None, target_bir_lowering=False, name="dflstm")

    u_d = nc.dram_tensor("u_all", [TB, G4], bf16, kind="ExternalInput")
    w_d = nc.dram_tensor("wcat", [128, KT * G4], bf16, kind="ExternalInput")
    c_d = nc.dram_tensor("ctt", [128, NM * KT * MT], bf16, kind="ExternalInput")
    cb_d = nc.dram_tensor("cbias", [MT, NM], f32, kind="ExternalInput")
    out_d = nc.dram_tensor("out", [VS, TB], f32, kind="ExternalOutput")

    with tile.TileContext(nc) as tc:
        with (
            tc.tile_pool(name="wpool", bufs=1) as wpool,
            tc.tile_pool(name="hpool", bufs=1) as hpool,
            tc.tile_pool(name="spool", bufs=1) as spool,
            tc.tile_pool(name="upool", bufs=2) as upool,
            tc.tile_pool(name="gpool", bufs=10) as gpool,
            tc.tile_pool(name="tpool", bufs=2) as tpool,
            tc.tile_pool(name="hspool", bufs=2) as hspool,
            tc.tile_pool(name="ctpool", bufs=3) as ctpool,
            tc.tile_pool(name="opool", bufs=2) as opool,
            tc.tile_pool(name="cpool", bufs=1) as cpool,
            tc.tile_pool(name="psg", bufs=3, space="PSUM") as psg,
            tc.tile_pool(name="pst", bufs=2, space="PSUM") as pst,
            tc.tile_pool(name="psp", bufs=2, space="PSUM") as psp,
        ):
            ident = cpool.tile([B, B], f32)
            make_identity(nc, ident[:])

            def body(_iv=None):
                # resident weights
                wcat_sb = wpool.tile([128, KT * G4], bf16)
                for k in range(KT):
                    nc.sync.dma_start(
                        wcat_sb[:, k * G4:(k + 1) * G4], w_d[:, k * G4:(k + 1) * G4]
                    )
                cb_sb = cpool.tile([MT, NM], f32)
                nc.sync.dma_start(cb_sb[:], cb_d[:])

                # hiddens^T slab: [128, (slot s=0..T-1, k=0..7, c=0..63)] bf16
                hidT = hpool.tile([128, T * H // 2 * 2], bf16)  # T*512 cols
                c_sb = spool.tile([B, H], f32)

                for t in range(T):
                    u_sb = upool.tile([B, G4], bf16)
                    nc.sync.dma_start(u_sb[:], u_d[t * B:(t + 1) * B, :])

                    chunks = []
                    for n in range(8):
                        csl = slice(n * 512, (n + 1) * 512)
                        fn = Tanh if n >= 6 else Sig
                        g_sb = gpool.tile([B, 512], f32)
                        if t == 0:
                            nc.scalar.activation(g_sb[:], u_sb[:, csl], fn)
                        else:
                            ps = psg.tile([B, 512], f32, space="PSUM")
                            for k in range(KT):
                                nc.tensor.matmul(
                                    ps[:],
                                    lhsT=hidT[:, (t - 1) * 512 + k * 64:
                                              (t - 1) * 512 + k * 64 + 64],
                                    rhs=wcat_sb[:, k * G4 + n * 512:
                                                k * G4 + n * 512 + 512],
                                    start=(k == 0),
                                    stop=(k == KT - 1),
                                )
                            nc.vector.tensor_add(g_sb[:], ps[:], u_sb[:, csl])
                            nc.scalar.activation(g_sb[:], g_sb[:], fn)
                        chunks.append(g_sb)

                    h_sb = hspool.tile([B, H], f32)
                    for g in range(2):
                        sl = slice(g * 512, g * 512 + 512)
                        i_g, f_g, o_g, ct_g = (chunks[0 + g], chunks[2 + g],
                                               chunks[4 + g], chunks[6 + g])
                        if t == 0:
                            nc.vector.tensor_mul(c_sb[:, sl], i_g[:], ct_g[:])
                        else:
                            tmp = tpool.tile([B, 512], f32)
                            nc.vector.tensor_mul(tmp[:], i_g[:], ct_g[:])
                            nc.vector.tensor_mul(c_sb[:, sl], f_g[:], c_sb[:, sl])
                            nc.vector.tensor_add(c_sb[:, sl], c_sb[:, sl], tmp[:])
                        nc.vector.tensor_mul(h_sb[:, sl], o_g[:], c_sb[:, sl])

                    # transpose h [64, 1024] -> hidT slot t ([128, 512], k-blocks)
                    ps_tr = pst.tile([128, 512], f32, space="PSUM")
                    for k in range(KT):
                        nc.tensor.transpose(
                            ps_tr[:, k * 64:(k + 1) * 64],
                            h_sb[:, k * 128:(k + 1) * 128],
                            ident[:],
                        )
                    nc.vector.tensor_copy(
                        hidT[:, t * 512:(t + 1) * 512], ps_tr[:]
                    )

                    # projection for completed 8-step chunk
                    if (t + 1) % TCH == 0:
                        tau = t // TCH
                        hview = hidT[:].rearrange(
                            "p (s k c) -> p s k c", k=KT, c=64
                        )
                        for m in range(NM):
                            cts = ctpool.tile([128, KT * MT], bf16)
                            nc.sync.dma_start(
                                cts[:],
                                c_d[:, m * KT * MT:(m + 1) * KT * MT],
                            )
                            pp = psp.tile([MT, 512], f32, space="PSUM")
                            for k in range(KT):
                                nc.tensor.matmul(
                                    pp[:],
                                    lhsT=cts[:, k * MT:(k + 1) * MT],
                                    rhs=hview[:, tau * TCH:(tau + 1) * TCH, k, :],
                                    start=(k == 0),
                                    stop=(k == KT - 1),
                                )
                            ob = opool.tile([MT, 512], f32)
                            nc.scalar.activation(
                                ob[:], pp[:], Ident, bias=cb_sb[:, m:m + 1]
                            )
                            nc.sync.dma_start(
                                out_d[m * MT:(m + 1) * MT,
                                      tau * 512:(tau + 1) * 512],
                                ob[:],
                            )

            if reps > 1:
                with tc.For_i(0, reps, 1):
                    body()
            else:
                body()

    nc.compile()
    return nc


# ---------------------------------------------------------------- host prep
def _np(a):
    return np.asarray(a, dtype=np.float32)


def prepare_inputs(params, captions):
    """Fold the affine pre-chain on host; build device input maps."""
    cap = np.asarray(captions).astype(np.int64)          # [B, T]
    B_emb = _np(params["B_emb"])                         # [V, E]

    Ms, bs = [], []
    for g in "ifoc":
        Vw, Vb = (_np(x) for x in params[f"V_{g}"])
        Sw, Sb = (_np(x) for x in params[f"Sf_{g}"])
        Uw, Ub = (_np(x) for x in params[f"U_{g}"])
        Ww, Wb = (_np(x) for x in params[f"W_{g}"])
        M = Uw @ Sw @ Vw                                 # [H, E]
        bias = Vb @ Sw.T @ Uw.T + Sb @ Uw.T + Ub + Wb    # [H]
        Ms.append(M)
        bs.append(bias)
    Mcat = np.concatenate(Ms, axis=0)                    # [4H, E]
    bcat = np.concatenate(bs, axis=0)                    # [4H]

    # u rows in time-major order r = t*B + b
    flat = cap.T.ravel()                                 # [T*B]
    uniq, inv = np.unique(flat, return_inverse=True)
    u_uniq = B_emb[uniq] @ Mcat.T + bcat                 # [NU, 4H] fp32
    u_all = u_uniq[inv].astype(BF16)                     # [T*B, 4H]

    # Wcat^T tiled: [128, (k, 4H)]
    Wcat = np.concatenate([_np(params[f"W_{g}"][0]) for g in "ifoc"], axis=0)
    wcatT = np.ascontiguousarray(
        Wcat.T.reshape(KT, 128, G4).transpose(1, 0, 2).reshape(128, KT * G4)
    ).astype(BF16)

    Cw, Cb = (_np(x) for x in params["C"])               # [V, H], [V]
    in_maps = []
    for c in range(NCORES):
        Cs = Cw[c * VS:(c + 1) * VS]                     # [VS, H]
        CsT = Cs.T                                       # [H, VS]
        ctt = np.ascontiguousarray(
            CsT.reshape(KT, 128, NM, MT).transpose(1, 2, 0, 3)
            .reshape(128, NM * KT * MT)
        ).astype(BF16)
        cbias = np.ascontiguousarray(
            Cb[c * VS:(c + 1) * VS].reshape(NM, MT).T
        ).astype(np.float32)
        in_maps.append({
            "u_all": u_all, "wcat": wcatT, "ctt": ctt, "cbias": cbias,
        })
    return in_maps


# ---------------------------------------------------------------- runner
class SpmdRunner:
    """Compile once; run N times on device-resident inputs (no donation)."""

    def __init__(self, nc, n_cores=NCORES):
        from jax.sharding import Mesh, PartitionSpec, NamedSharding
        from jax.experimental.shard_map import shard_map

        bass2jax.install_neuronx_cc_hook()
        self.nc = nc
        self.n_cores = n_cores
        partition_name = (
            nc.partition_id_tensor.name if nc.partition_id_tensor else None
        )
        in_names, out_names, out_avals, zero_outs = [], [], [], []
        for alloc in nc.m.functions[0].allocations:
            if not isinstance(alloc, mybir.MemoryLocationSet):
                continue
            name = alloc.memorylocations[0].name
            if alloc.kind == "ExternalInput":
                if name != partition_name:
                    in_names.append(name)
            elif alloc.kind == "ExternalOutput":
                shape = tuple(alloc.tensor_shape)
                dtype = mybir.dt.np(alloc.dtype)
                out_names.append(name)
                out_avals.append(jax.core.ShapedArray(shape, dtype))
                zero_outs.append(np.zeros(shape, dtype))
        self.in_names, self.out_names = in_names, out_names
        self.out_avals, self.zero_outs = out_avals, zero_outs
        all_in = list(in_names) + list(out_names)
        if partition_name is not None:
            all_in.append(partition_name)

        def _body(*args):
            operands = list(args)
            if partition_name is not None:
                operands.append(bass2jax.partition_id_tensor())
            return tuple(bass2jax._bass_exec_p.bind(
                *operands,
                out_avals=tuple(out_avals),
                in_names=tuple(all_in),
                out_names=tuple(out_names),
                lowering_input_output_aliases=(),
                sim_require_finite=False,
                sim_require_nnan=False,
                nc=nc,
            ))

        devices = jax.devices()[:n_cores]
        mesh = Mesh(np.asarray(devices), ("core",))
        n_io = len(in_names) + len(out_names)
        self._shard = NamedSharding(mesh, PartitionSpec("core"))
        self.fn = jax.jit(shard_map(
            _body, mesh=mesh,
            in_specs=(PartitionSpec("core"),) * n_io,
            out_specs=(PartitionSpec("core"),) * len(out_names),
            check_rep=False,
        ))
        self._staged = None

    def stage(self, in_maps):
        n = self.n_cores
        concat_in = [
            jax.device_put(
                np.concatenate([np.asarray(in_maps[c][k]) for c in range(n)], 0),
                self._shard)
            for k in self.in_names
        ]
        concat_zero = [
            jax.device_put(np.zeros((n * z.shape[0], *z.shape[1:]), z.dtype),
                           self._shard)
            for z in self.zero_outs
        ]
        jax.block_until_ready(concat_in)
        jax.block_until_ready(concat_zero)
        self._staged = (concat_in, concat_zero)

    def run(self):
        concat_in, concat_zero = self._staged
        outs = self.fn(*concat_in, *concat_zero)
        jax.block_until_ready(outs)
        return outs

    def run_timed(self, n_calls=6):
        times = []
        outs = None
        for _ in range(n_calls):
            t0 = time.perf_counter()
            outs = self.run()
            times.append(time.perf_counter() - t0)
        return outs, min(times)

    def results(self, outs):
        n = self.n_cores
        return [
            {name: np.asarray(outs[i]).reshape(n, *self.out_avals[i].shape)[c]
             for i, name in enumerate(self.out_names)}
            for c in range(n)
        ]


def assemble_output(per_core):
    full = np.empty((TB, V), dtype=np.float32)
    for c in range(NCORES):
        full[:, c * VS:(c + 1) * VS] = per_core[c]["out"].T
    return full


# ---------------------------------------------------------------- entry point
_CACHE = {}


def _get_runner(reps):
    key = ("nc", reps)
    if key not in _CACHE:
        nc = build_nc(reps=reps)
        _CACHE[key] = SpmdRunner(nc)
    return _CACHE[key]


def kernel(params, captions):
    runner = _get_runner(int(os.environ.get("DFL_REPS", "1")))
    in_maps = prepare_inputs(params, captions)
    runner.stage(in_maps)
    outs = runner.run()
    return assemble_output(runner.results(outs))
